# revision 1
# baseline (speedup 1.0000x reference)
"""Causal multi-head attention (B=4, T=2048, C=1024, 16 heads) on 8 TRN2 NeuronCores.

Sharding: data-parallel over (batch, q-chunk-pair). Core 2*b+h handles batch b
and two 512-row q-chunks chosen so every core runs an identical program:
  core (b,0): chunk A = rows [0:512]     (program kv extent 1024)
              chunk B = rows [1536:2048] (program kv extent 2048)
  core (b,1): chunk A = rows [512:1024]  (kv extent 1024)
              chunk B = rows [1024:1536] (kv extent 2048, data extent 1536)
Causality inside the rectangles is enforced with per-core {0,1} multiplicative
masks supplied as data, so the instruction stream is core-independent (SPMD).

Everything on-device lives transposed ([feature, token]): softmax denominators
come out of the TensorEngine via a ones-column appended to V, and no on-device
transposes are needed; the host transposes x in and the output out.

Inputs/weights/activations are bf16 (PE at full rate, fp32 PSUM accumulation);
the l/normalization path is fp32. Score matmuls for a head pair run on PE
row-groups 0-63 / 64-127 concurrently (contract dim is 64).

The emission order interleaves PE-heavy projection work into the ACT-bound
attention phases: K/V projections for kv [1024:2048] and the chunk-B Q
projection are spread between chunk-A head pairs; the chunk-A output
projection is spread between chunk-B head pairs.
"""

import numpy as np
import ml_dtypes

B, T, C, NH, D = 4, 2048, 1024, 16, 64
P = 128
CH = 512                # q-chunk size
KV_EXT = (1024, 2048)   # program kv extent for chunk A / chunk B

_CACHE = {}


def _build():
    import concourse.bacc as bacc
    import concourse.tile as tile
    import concourse.mybir as mybir
    from concourse.bass import ts, ds

    f32 = mybir.dt.float32
    bf16 = mybir.dt.bfloat16
    ID = mybir.ActivationFunctionType.Identity
    EXP = mybir.ActivationFunctionType.Exp
    COPY = mybir.ActivationFunctionType.Copy
    MUL = mybir.AluOpType.mult
    ADD = mybir.AluOpType.add

    nc = bacc.Bacc("TRN2", target_bir_lowering=False, debug=False, num_devices=8)

    def din(name, shape, dt=bf16):
        return nc.dram_tensor(name, list(shape), dt, kind="ExternalInput").ap()

    xqT = din("xqT", (C, 2 * CH))    # x^T, this core's q rows (A then B)
    xkvT = din("xkvT", (C, T))       # x^T, full batch (for K/V)
    wqT = din("wqT", (C, C))         # (Wq/8)^T
    wkT = din("wkT", (C, C))
    wvT = din("wvT", (C, C))
    woT = din("woT", (C, C))
    bq = din("bq", (P, C // P), f32)     # bq/8, chunked [128, 8]
    bk = din("bk", (P, C // P), f32)
    bo = din("bo", (P, C // P), f32)
    maskA = din("maskA", (KV_EXT[0], CH))     # {0,1}, [kv, q] chunk A
    maskB = din("maskB", (1024, CH))          # chunk B, kv in [1024:2048]
    out = nc.dram_tensor("out", [C, 2 * CH], f32, kind="ExternalOutput").ap()

    KC = C // P        # 8 contraction chunks
    NT = T // P        # 16 kv chunks of the full batch

    wq_v = wqT.rearrange("(ko p) m -> p ko m", p=P)
    wk_v = wkT.rearrange("(ko p) m -> p ko m", p=P)
    wo_v = woT.rearrange("(ko p) m -> p ko m", p=P)
    wv_v = wvT.rearrange("(ko p) c -> p ko c", p=P)
    xkv_v = xkvT.rearrange("(ko p) t -> p ko t", p=P)
    xq_v = xqT.rearrange("(ko p) t -> p ko t", p=P)
    maskA_v = maskA.rearrange("(ko p) q -> p ko q", p=P)
    maskB_v = maskB.rearrange("(ko p) q -> p ko q", p=P)

    from contextlib import ExitStack
    with ExitStack() as ctx:
        tc = ctx.enter_context(tile.TileContext(nc))

        consts = ctx.enter_context(tc.tile_pool(name="consts", bufs=1))
        big = ctx.enter_context(tc.tile_pool(name="big", bufs=1))
        wpool = ctx.enter_context(tc.tile_pool(name="w", bufs=2))
        xkpool = ctx.enter_context(tc.tile_pool(name="xk", bufs=2))
        xvpool = ctx.enter_context(tc.tile_pool(name="xv", bufs=2))
        qpool = ctx.enter_context(tc.tile_pool(name="q", bufs=1))
        mpool = ctx.enter_context(tc.tile_pool(name="m", bufs=1))
        xqpool = ctx.enter_context(tc.tile_pool(name="xq", bufs=1))
        ptpool = ctx.enter_context(tc.tile_pool(name="pt", bufs=4))
        ctxpool = ctx.enter_context(tc.tile_pool(name="ctx", bufs=1))
        lpool = ctx.enter_context(tc.tile_pool(name="l", bufs=2))
        l0pool = ctx.enter_context(tc.tile_pool(name="l0", bufs=2))
        lbpool = ctx.enter_context(tc.tile_pool(name="lb", bufs=2))
        cspool = ctx.enter_context(tc.tile_pool(name="cs", bufs=3))
        opool = ctx.enter_context(tc.tile_pool(name="o", bufs=2))
        psumP = ctx.enter_context(tc.tile_pool(name="psumP", bufs=2, space="PSUM"))
        psumS = ctx.enter_context(tc.tile_pool(name="psumS", bufs=2, space="PSUM"))
        psumX = ctx.enter_context(tc.tile_pool(name="psumX", bufs=2, space="PSUM"))

        bq_sb = consts.tile([P, KC], f32)
        bk_sb = consts.tile([P, KC], f32)
        bo_sb = consts.tile([P, KC], f32)
        nc.sync.dma_start(bq_sb[:], bq)
        nc.sync.dma_start(bk_sb[:], bk)
        nc.sync.dma_start(bo_sb[:], bo)

        KT_sb = big.tile([P, KC, T], bf16)          # K^T  [c, t]
        V_sb = big.tile([P, NT, NH, D + 1], bf16)   # V + ones col per chunk/head
        nc.vector.memset(V_sb[:, :, :, D : D + 1], 1.0)
        wvt0 = big.tile([P, KC, CH], bf16)          # Wv^T halves, resident
        wvt1 = big.tile([P, KC, CH], bf16)
        nc.sync.dma_start(wvt0[:], wv_v[:, :, 0:CH])
        nc.sync.dma_start(wvt1[:], wv_v[:, :, CH:C])
        wvt = [wvt0, wvt1]

        # ---------- emission helpers ----------
        XK = {}

        def kt_proj(ft, m0, m1):
            """KT[:, m0:m1, 512*ft:...] from a cached xk tile."""
            if ft not in XK:
                XK[ft] = xkpool.tile([P, KC, 512], bf16, tag="xk",
                                     name=f"xk{ft}")
                nc.sync.dma_start(XK[ft][:], xkv_v[:, :, ds(512 * ft, 512)])
            xk = XK[ft]
            for m in range(m0, m1):
                wt = wpool.tile([P, KC, P], bf16, tag="w", name=f"wk{ft}{m}")
                nc.sync.dma_start(wt[:], wk_v[:, :, ts(m, P)])
                ps = psumP.tile([P, 512], f32, tag="psP", name=f"pk{ft}{m}")
                for k in range(KC):
                    nc.tensor.matmul(ps[:], wt[:, k, :], xk[:, k, :],
                                     start=(k == 0), stop=(k == KC - 1))
                nc.scalar.activation(KT_sb[:, m, ds(512 * ft, 512)], ps[:],
                                     ID, bias=bk_sb[:, m : m + 1])

        def v_proj(i):
            """V rows [128*i : 128*(i+1)], all channels."""
            xv = xvpool.tile([P, KC, P], bf16, tag="xv", name=f"xv{i}")
            nc.sync.dma_start(xv[:], xkv_v[:, :, ts(i, P)])
            for chh in range(2):
                ps = psumP.tile([P, 512], f32, tag="psP", name=f"pv{i}{chh}")
                for k in range(KC):
                    nc.tensor.matmul(ps[:], xv[:, k, :], wvt[chh][:, k, :],
                                     start=(k == 0), stop=(k == KC - 1))
                nc.scalar.activation(
                    V_sb[:, i, ds(8 * chh, 8), 0:D],
                    ps.rearrange("p (h d) -> p h d", d=D), COPY)

        QT = {}

        def q_proj(qc, m0, m1):
            if qc not in QT:
                QT[qc] = qpool.tile([P, KC, CH], bf16, tag=f"qt{qc}",
                                    name=f"qt{qc}")
            if ("xq", qc) not in QT:
                QT[("xq", qc)] = xqpool.tile([P, KC, CH], bf16, tag="xq",
                                             name=f"xq{qc}")
                nc.sync.dma_start(QT[("xq", qc)][:],
                                  xq_v[:, :, ds(CH * qc, CH)])
            xq = QT[("xq", qc)]
            for m in range(m0, m1):
                wt = wpool.tile([P, KC, P], bf16, tag="w", name=f"wq{qc}{m}")
                nc.sync.dma_start(wt[:], wq_v[:, :, ts(m, P)])
                ps = psumP.tile([P, CH], f32, tag="psP", name=f"pq{qc}{m}")
                for k in range(KC):
                    nc.tensor.matmul(ps[:], wt[:, k, :], xq[:, k, :],
                                     start=(k == 0), stop=(k == KC - 1))
                nc.scalar.activation(QT[qc][:, m, :], ps[:], ID,
                                     bias=bq_sb[:, m : m + 1])

        def o_proj(qc, ctxT, m):
            wt = wpool.tile([P, KC, P], bf16, tag="w", name=f"wo{qc}{m}")
            nc.sync.dma_start(wt[:], wo_v[:, :, ts(m, P)])
            ps = psumP.tile([P, CH], f32, tag="psP", name=f"po{qc}{m}")
            for k in range(KC):
                nc.tensor.matmul(ps[:], wt[:, k, :], ctxT[:, k, :],
                                 start=(k == 0), stop=(k == KC - 1))
            o_sb = opool.tile([P, CH], f32, tag="o", name=f"o{qc}{m}")
            nc.scalar.activation(o_sb[:], ps[:], ID, bias=bo_sb[:, m : m + 1])
            nc.sync.dma_start(out[ts(m, P), ds(CH * qc, CH)], o_sb[:])

        def attn_pair(qc, hp, msk, ctxT):
            E = KV_EXT[qc]
            NKV = E // P
            ctx_ps = [psumX.tile([P, CH], f32, tag="psX", name=f"psX{qc}{hp}{i}")
                      for i in range(2)]
            for kvc in range(NKV):
                st = psumS.tile([P, 2, CH], f32, tag="psS",
                                name=f"psS{qc}{hp}{kvc}")
                for hh in range(2):
                    # contract dim 64 at PE row-group 64*hh: the two heads'
                    # score matmuls run concurrently in the array
                    nc.tensor.matmul(
                        st[:, hh, :],
                        KT_sb[ds(64 * hh, 64), hp, ds(P * kvc, P)],
                        QT[qc][ds(64 * hh, 64), hp, :],
                        start=True, stop=True)
                pt = ptpool.tile([P, 2, CH], bf16, tag="pt",
                                 name=f"pt{qc}{hp}{kvc}")
                nc.scalar.activation(pt[:], st[:], EXP)
                mi = kvc if qc == 0 else kvc - NKV // 2
                if mi >= 0:   # causal mask (chunk A: all; chunk B: kv >= 1024)
                    nc.vector.tensor_tensor(
                        pt[:], pt[:],
                        msk[:, mi : mi + 1, :].to_broadcast((P, 2, CH)), MUL)
                for hh in range(2):
                    nc.tensor.matmul(
                        ctx_ps[hh][0 : D + 1, :],
                        V_sb[:, kvc, 2 * hp + hh, :],
                        pt[:, hh, :],
                        start=(kvc == 0), stop=(kvc == NKV - 1))
            # Epilogue. Free the PSUM banks fast (reciprocal of row D + DVE
            # copy of rows [0:D) to SBUF); the 1/l row is hopped to physical
            # partition 0 (the only one HW partition_broadcast reads) on the
            # GpSimd DMA queue, broadcast on GpSimd, normalized on DVE, and
            # partition-remapped into ctxT with a GpSimd-queued DMA.
            cs = []
            for hh in range(2):
                l_sb = lpool.tile([P, CH], f32, tag="l", name=f"l{qc}{hp}{hh}")
                nc.vector.reciprocal(l_sb[D : D + 1, :],
                                     ctx_ps[hh][D : D + 1, :])
                l0 = l0pool.tile([1, CH], f32, tag="l0", name=f"l0{qc}{hp}{hh}")
                nc.gpsimd.dma_start(l0[:], l_sb[D : D + 1, :])
                c_scr = cspool.tile([P, CH], f32, tag="cs",
                                    name=f"cs{qc}{hp}{hh}")
                nc.vector.tensor_copy(c_scr[0:D, :], ctx_ps[hh][0:D, :])
                cs.append((l0, c_scr))
            for hh in range(2):
                l0, c_scr = cs[hh]
                linv = lbpool.tile([P, CH], f32, tag="lb", name=f"lb{qc}{hp}{hh}")
                nc.gpsimd.partition_broadcast(linv[0:D, :], l0[:], channels=D)
                if hh == 0:
                    nc.vector.tensor_tensor(ctxT[0:D, hp, :], c_scr[0:D, :],
                                            linv[0:D, :], MUL)
                else:
                    c2 = cspool.tile([P, CH], bf16, tag="cs2",
                                     name=f"cs2{qc}{hp}")
                    nc.vector.tensor_tensor(c2[0:D, :], c_scr[0:D, :],
                                            linv[0:D, :], MUL)
                    nc.gpsimd.dma_start(ctxT[ds(64, 64), hp, :], c2[0:D, :])

        # ---------- emission schedule ----------
        # prologue: K/V for kv [0:1024], Q for chunk A
        for ft in range(2):
            kt_proj(ft, 0, 4); kt_proj(ft, 4, 8)
        for i in range(8):
            v_proj(i)
        q_proj(0, 0, 4); q_proj(0, 4, 8)

        mskA = mpool.tile([P, KC, CH], bf16, tag="mask", name="mA")
        nc.sync.dma_start(mskA[:], maskA_v)

        # chunk A attention, with kv[1024:2048] K/V projections and the
        # chunk-B Q projection interleaved as PE filler
        ctxT_A = ctxpool.tile([P, KC, CH], bf16, tag="ctxA", name="ctxA")
        fillers = ([lambda ft=ft, m0=m0: kt_proj(ft, m0, m0 + 4)
                    for ft in (2, 3) for m0 in (0, 4)]
                   + [lambda i=i: v_proj(i) for i in range(8, 16)]
                   + [lambda m0=m0: q_proj(1, m0, m0 + 4) for m0 in (0, 4)])
        fi = 0
        for hp in range(NH // 2):
            attn_pair(0, hp, mskA, ctxT_A)
            take = (len(fillers) - fi + (NH // 2 - hp) - 1) // (NH // 2 - hp)
            for _ in range(take):
                if fi < len(fillers):
                    fillers[fi](); fi += 1
        while fi < len(fillers):
            fillers[fi](); fi += 1

        # chunk B attention, with chunk-A output projection interleaved
        mskB = mpool.tile([P, KC, CH], bf16, tag="mask", name="mB")
        nc.sync.dma_start(mskB[:], maskB_v)
        ctxT_B = ctxpool.tile([P, KC, CH], bf16, tag="ctxB", name="ctxB")
        for hp in range(NH // 2):
            attn_pair(1, hp, mskB, ctxT_B)
            o_proj(0, ctxT_A, hp)
        for m in range(NH // 2, KC):
            o_proj(0, ctxT_A, m)
        for m in range(KC):
            o_proj(1, ctxT_B, m)

    nc.compile()
    return nc


def _shard_inputs(x, Wq, bq, bk_, bv, bo, WqT, WkT, WvT, WoT):
    """Build the 8 per-core input maps (bf16 data tensors, fp32 biases).

    bv is folded into the output-projection bias: ctx = ctx0 + 1*bv^T, so
    out = ctx0 @ Wo^T + (bo + Wo @ bv)."""
    bf = ml_dtypes.bfloat16
    in_maps = []
    rows = {0: (np.arange(0, 512), np.arange(1536, 2048)),
            1: (np.arange(512, 1024), np.arange(1024, 1536))}
    kv = np.arange(T)
    bq8 = np.ascontiguousarray((bq / 8.0).reshape(C // P, P).T)
    bk8 = np.ascontiguousarray(bk_.reshape(C // P, P).T)
    bo_f = bo + WoT.T @ bv
    bo8 = np.ascontiguousarray(bo_f.reshape(C // P, P).T)
    wq16, wk16 = WqT.astype(bf), WkT.astype(bf)
    wv16, wo16 = WvT.astype(bf), WoT.astype(bf)
    for b in range(B):
        xT = np.ascontiguousarray(x[b].T).astype(bf)     # (C, T)
        for h in range(2):
            qA, qB = rows[h]
            xqT = np.ascontiguousarray(xT[:, np.concatenate([qA, qB])])
            mA = (kv[:1024, None] <= qA[None, :]).astype(bf)
            mB = (kv[1024:, None] <= qB[None, :]).astype(bf)
            in_maps.append({
                "xqT": xqT, "xkvT": xT,
                "wqT": wq16, "wkT": wk16, "wvT": wv16, "woT": wo16,
                "bq": bq8, "bk": bk8, "bo": bo8,
                "maskA": np.ascontiguousarray(mA),
                "maskB": np.ascontiguousarray(mB),
            })
    return in_maps


def kernel(x, Wq, bq, Wk, bk, Wv, bv, Wo, bo):
    from concourse.bass_utils import run_bass_kernel_spmd

    x = np.asarray(x, np.float32)
    Wq = np.asarray(Wq, np.float32); bq = np.asarray(bq, np.float32)
    Wk = np.asarray(Wk, np.float32); bk = np.asarray(bk, np.float32)
    Wv = np.asarray(Wv, np.float32); bv = np.asarray(bv, np.float32)
    Wo = np.asarray(Wo, np.float32); bo = np.asarray(bo, np.float32)

    if "nc" not in _CACHE:
        _CACHE["nc"] = _build()
    nc = _CACHE["nc"]

    WqT = np.ascontiguousarray(Wq.T / 8.0)
    WkT = np.ascontiguousarray(Wk.T)
    WvT = np.ascontiguousarray(Wv.T)
    WoT = np.ascontiguousarray(Wo.T)
    in_maps = _shard_inputs(x, Wq, bq, bk, bv, bo, WqT, WkT, WvT, WoT)

    res = run_bass_kernel_spmd(nc, in_maps, core_ids=list(range(8)))
    outf = np.empty((B, T, C), np.float32)
    rows = {0: (np.arange(0, 512), np.arange(1536, 2048)),
            1: (np.arange(512, 1024), np.arange(1024, 1536))}
    for b in range(B):
        for h in range(2):
            o = res.results[2 * b + h]["out"]          # (C, 1024) transposed
            qA, qB = rows[h]
            outf[b, qA, :] = o[:, :512].T
            outf[b, qB, :] = o[:, 512:].T
    return outf



# revision 9
# speedup vs baseline: 1.1052x; 1.1052x over previous
"""Causal multi-head attention (B=4, T=2048, C=1024, 16 heads) on 8 TRN2 cores.

Tensor-parallel over heads: core c owns heads 2c, 2c+1 (128 features) and
computes Q/K/V projections + attention for those heads over ALL 4 batches.
Causal work uses exact extents (q-chunk 512, kv extent 512*(qc+1)); only the
diagonal 512x512 block needs a {0,1} mask, which is identical for every
(batch, chunk) so a single [128,4,512] mask tile serves the whole kernel.
Every core runs the same instruction stream (SPMD) - only weight slices
differ - so no per-core masking of fully-dead blocks is needed.

After attention, a per-batch 512KB AllToAll redistributes context from
feature-sharded to token-sharded; each core then runs the output projection
for its 256-token slice of each batch (contract over the full 1024 features).

On-device layout is transposed ([feature, token]) except V (token-major, as
the ctx matmul's stationary operand). Softmax denominators come from a ones
column at position 0 of V (l lands at PSUM partition 0); 1/l is broadcast
across partitions with a contract-1 PE matmul instead of gpsimd.

bf16 data path with fp32 PSUM; score scale 1/8 folded into Wq/bq; V bias
folded into the output bias (bo + Wo@bv). Projections for batch b+1 and the
output projection for batch b-1 are interleaved into batch b's attention as
PE filler.
"""

import numpy as np
import ml_dtypes

B, T, C, NH, D = 4, 2048, 1024, 16, 64
P = 128
KC = C // P          # 8 contraction chunks
CH = 512             # q-chunk / projection token-chunk size
NCHUNK = T // CH     # 4 chunks per batch
TS = 256             # per-core token slice of each batch (T/8)

_CACHE = {}


def _build():
    import concourse.bacc as bacc
    import concourse.tile as tile
    import concourse.mybir as mybir
    from concourse.bass import ts, ds

    f32 = mybir.dt.float32
    bf16 = mybir.dt.bfloat16
    EXP = mybir.ActivationFunctionType.Exp
    MUL = mybir.AluOpType.mult
    ADD = mybir.AluOpType.add

    nc = bacc.Bacc("TRN2", target_bir_lowering=False, debug=False, num_devices=8)

    def din(name, shape, dt=bf16):
        return nc.dram_tensor(name, list(shape), dt, kind="ExternalInput").ap()

    xT = din("xT", (C, B * T))          # x^T, all batches (token = b*T + t)
    wq = din("wq", (C, P))              # (Wq/8)^T columns for this core's heads
    wk = din("wk", (C, P))
    wv = din("wv", (C, P))
    wo = din("wo", (C, C))              # full Wo^T
    bq = din("bq", (P, 1), f32)         # bq/8 slice
    bk = din("bk", (P, 1), f32)
    bo = din("bo", (P, KC), f32)        # (bo + Wo@bv) chunked [128, 8]
    mask = din("mask", (P, 4, CH))      # diag-block causal mask {0,1}
    out = nc.dram_tensor("out", [C, B * TS], f32, kind="ExternalOutput").ap()

    xT_v = xT.rearrange("(k p) t -> p k t", p=P)
    wq_v = wq.rearrange("(k p) m -> p k m", p=P)
    wk_v = wk.rearrange("(k p) m -> p k m", p=P)
    wv_v = wv.rearrange("(k p) m -> p k m", p=P)
    wo_v = wo.rearrange("(k p) m -> p k m", p=P)
    out_v = out.rearrange("(k p) t -> p k t", p=P)

    NBLK = T // P      # 16 kv blocks of 128 per batch

    from contextlib import ExitStack
    with ExitStack() as ctx:
        tc = ctx.enter_context(tile.TileContext(nc))

        consts = ctx.enter_context(tc.tile_pool(name="consts", bufs=1))
        wpool = ctx.enter_context(tc.tile_pool(name="w", bufs=1))
        xpool = ctx.enter_context(tc.tile_pool(name="x", bufs=3))
        qkpool = ctx.enter_context(tc.tile_pool(name="qk", bufs=2))
        vpool = ctx.enter_context(tc.tile_pool(name="v", bufs=2))
        ptpool = ctx.enter_context(tc.tile_pool(name="pt", bufs=4))
        lpool = ctx.enter_context(tc.tile_pool(name="l", bufs=4))
        cspool = ctx.enter_context(tc.tile_pool(name="cs", bufs=4))
        gpool = ctx.enter_context(tc.tile_pool(name="g", bufs=2))
        opool = ctx.enter_context(tc.tile_pool(name="o", bufs=2))
        psum = ctx.enter_context(tc.tile_pool(name="psum", bufs=2, space="PSUM"))
        dram = ctx.enter_context(tc.tile_pool(name="dram", bufs=2, space="DRAM"))

        # ---- constants ----
        bq_sb = consts.tile([P, 1], f32)
        bk_sb = consts.tile([P, 1], f32)
        bo_sb = consts.tile([P, KC], f32)
        msk_sb = consts.tile([P, 4, CH], bf16)
        nc.sync.dma_start(bq_sb[:], bq)
        nc.sync.dma_start(bk_sb[:], bk)
        nc.sync.dma_start(bo_sb[:], bo)
        nc.sync.dma_start(msk_sb[:], mask)
        # broadcast stationary [1,1,...,1,0] living at partition 64 (so it can
        # pair with the 1/l row, which lands at PSUM partition 64)
        pvec = consts.tile([D + 1, D + 1], bf16)
        nc.vector.memset(pvec[D : D + 1, 0:D], 1.0)
        nc.vector.memset(pvec[D : D + 1, D : D + 1], 0.0)

        wq_sb = wpool.tile([P, KC, P], bf16, tag="wq", name="wq_sb")
        wk_sb = wpool.tile([P, KC, P], bf16, tag="wk", name="wk_sb")
        wv_sb = wpool.tile([P, KC, P], bf16, tag="wv", name="wv_sb")
        wo_sb = wpool.tile([P, KC, C], bf16, tag="wo", name="wo_sb")
        nc.sync.dma_start(wq_sb[:], wq_v)
        nc.sync.dma_start(wk_sb[:], wk_v)
        nc.sync.dma_start(wv_sb[:], wv_v)
        nc.sync.dma_start(wo_sb[:], wo_v)

        # ---- per-batch state ----
        QT = {}   # b -> [128, T] bf16 (2 heads x 64d on partitions)
        KT = {}
        VS = {}   # b -> [128, NBLK, 2, 65] token-major V (+ones col at 0)
        GC = {}   # b -> [128, KC, TS] gathered full-feature ctx
        BIN = {}  # b -> DRAM alltoall input bounce
        BOUT = {}

        def proj_chunk_q(b, t):
            """Tokens [CH*t, CH*t+CH) of batch b -> QT/KT chunk + V blocks."""
            if b not in QT:
                QT[b] = qkpool.tile([P, T], bf16, tag="qt", name=f"qt{b}")
                KT[b] = qkpool.tile([P, T], bf16, tag="kt", name=f"kt{b}")
                VS[b] = vpool.tile([P, NBLK, 2, D + 1], bf16, tag="v",
                                   name=f"v{b}")
                nc.vector.memset(VS[b][:, :, :, D : D + 1], 1.0)
            xt = xpool.tile([P, KC, CH], bf16, tag="x", name=f"x{b}{t}")
            nc.sync.dma_start(xt[:], xT_v[:, :, ds(T * b + CH * t, CH)])
            XT[(b, t)] = xt
            ps = psum.tile([P, CH], f32, tag="mix", name=f"pq{b}{t}")
            for k in range(KC):
                nc.tensor.matmul(ps[:], wq_sb[:, k, :], xt[:, k, :],
                                 start=(k == 0), stop=(k == KC - 1))
            nc.vector.tensor_tensor(QT[b][:, ds(CH * t, CH)], ps[:],
                                    bq_sb.to_broadcast((P, CH)), ADD)

        def proj_chunk_k(b, t):
            xt = XT[(b, t)]
            ps = psum.tile([P, CH], f32, tag="mix", name=f"pk{b}{t}")
            for k in range(KC):
                nc.tensor.matmul(ps[:], wk_sb[:, k, :], xt[:, k, :],
                                 start=(k == 0), stop=(k == KC - 1))
            nc.vector.tensor_tensor(KT[b][:, ds(CH * t, CH)], ps[:],
                                    bk_sb.to_broadcast((P, CH)), ADD)

        def proj_chunk_v(b, t):
            xt = XT.pop((b, t))
            for i in range(4):
                blk = 4 * t + i
                ps = psum.tile([P, P], f32, tag="mix", name=f"pv{b}{blk}")
                for k in range(KC):
                    nc.tensor.matmul(ps[:], xt[:, k, ts(i, P)], wv_sb[:, k, :],
                                     start=(k == 0), stop=(k == KC - 1))
                nc.vector.tensor_copy(
                    VS[b][:, blk, :, 0:D],
                    ps.rearrange("t (h d) -> t h d", d=D))

        XT = {}

        def oproj_unit(b, fc):
            """Output features [128*fc, 128*fc+128) for batch b's token slice."""
            g = GC[b]
            ps = psum.tile([P, TS], f32, tag="mix", name=f"po{b}{fc}")
            for k in range(KC):
                nc.tensor.matmul(ps[:], wo_sb[:, k, ts(fc, P)], g[:, k, :],
                                 start=(k == 0), stop=(k == KC - 1))
            st = opool.tile([P, TS], f32, tag="o", name=f"o{b}{fc}")
            nc.vector.tensor_tensor(st[:], ps[:],
                                    bo_sb[:, fc : fc + 1].to_broadcast((P, TS)),
                                    ADD)
            nc.sync.dma_start(out_v[:, fc, ds(TS * b, TS)], st[:])

        # ---- filler pump ----
        fillers = []
        fi = [0]

        def pump(n):
            k = 0
            while k < n and fi[0] < len(fillers):
                fillers[fi[0]]()
                fi[0] += 1
                k += 1

        def attn_chunk(b, qc):
            """Attention for q rows [CH*qc, CH*qc+CH), kv [0, CH*(qc+1))."""
            nkv = 4 * (qc + 1)
            ctx_ps = [psum.tile([D + 1, CH], f32, tag="ctx",
                                name=f"cx{b}{qc}{h}") for h in range(2)]
            for blk in range(nkv):
                st = psum.tile([P, 2, CH], f32, tag="st", name=f"st{b}{qc}{blk}")
                for hh in range(2):
                    nc.tensor.matmul(
                        st[:, hh, :],
                        KT[b][ds(D * hh, D), ds(P * blk, P)],
                        QT[b][ds(D * hh, D), ds(CH * qc, CH)],
                        start=True, stop=True)
                pt = ptpool.tile([P, 2, CH], bf16, tag="pt",
                                 name=f"pt{b}{qc}{blk}")
                nc.scalar.activation(pt[:], st[:], EXP)
                r = blk - 4 * qc
                if r >= 0:   # diagonal block: apply causal mask
                    nc.vector.tensor_tensor(
                        pt[:], pt[:],
                        msk_sb[:, r : r + 1, :].to_broadcast((P, 2, CH)), MUL)
                for hh in range(2):
                    nc.tensor.matmul(
                        ctx_ps[hh][:],
                        VS[b][:, blk, hh, :],
                        pt[:, hh, :],
                        start=(blk == 0), stop=(blk == nkv - 1))
                pump(1)
            # epilogue: ctx is psum rows 0..63, l at row 64 (partition-aligned)
            linv = [lpool.tile([D + 1, CH], bf16, tag="l", name=f"l{b}{qc}{h}")
                    for h in range(2)]
            with nc.allow_low_precision(reason="1/l in bf16; ~0.2% rel err"):
                for hh in range(2):
                    nc.vector.reciprocal(linv[hh][D : D + 1, :],
                                         ctx_ps[hh][D : D + 1, :])
            pump(1)
            bc = [psum.tile([D + 1, CH], f32, tag="mix", name=f"bc{b}{qc}{h}")
                  for h in range(2)]
            for hh in range(2):
                nc.tensor.matmul(bc[hh][:], pvec[D : D + 1, :],
                                 linv[hh][D : D + 1, :],
                                 start=True, stop=True)
            for hh in range(2):
                bcs = cspool.tile([D, CH], bf16, tag="bcs",
                                  name=f"bcs{b}{qc}{hh}")
                nc.vector.tensor_copy(bcs[:], bc[hh][0:D, :])
                cs = cspool.tile([D, CH], bf16, tag="cs",
                                 name=f"cs{b}{qc}{hh}")
                nc.vector.tensor_tensor(cs[:], ctx_ps[hh][0:D, :],
                                        bcs[:], MUL)
                # ship straight into the alltoall input bounce
                nc.sync.dma_start(
                    BIN[b].rearrange("j p t -> p j t")
                         [ds(D * hh, D), ds(2 * qc, 2), :],
                    cs.rearrange("p (j t) -> p j t", t=TS))

        def alltoall(b):
            BOUT[b] = dram.tile([8, P, TS], bf16, tag="cout", name=f"co{b}")
            nc.gpsimd.collective_compute(
                "AllToAll", mybir.AluOpType.bypass,
                replica_groups=[list(range(8))],
                ins=[BIN[b].opt()], outs=[BOUT[b].opt()])
            GC[b] = gpool.tile([P, KC, TS], bf16, tag="g", name=f"g{b}")
            nc.sync.dma_start(GC[b][:], BOUT[b].rearrange("s p t -> p s t"))

        # ---- emission schedule ----
        proj_chunk_q(0, 0); proj_chunk_k(0, 0); proj_chunk_v(0, 0)
        for b in range(B):
            BIN[b] = dram.tile([8, P, TS], bf16, tag="cin", name=f"ci{b}")
            for qc in range(NCHUNK):
                fillers.clear(); fi[0] = 0
                if b == 0 and qc < 3:
                    fillers += [lambda t=qc + 1: proj_chunk_q(0, t),
                                lambda t=qc + 1: proj_chunk_k(0, t),
                                lambda t=qc + 1: proj_chunk_v(0, t)]
                if b < 3:
                    fillers += [lambda t=qc, bb=b + 1: proj_chunk_q(bb, t),
                                lambda t=qc, bb=b + 1: proj_chunk_k(bb, t),
                                lambda t=qc, bb=b + 1: proj_chunk_v(bb, t)]
                if b > 0 and qc >= 2:
                    fillers += [lambda bb=b - 1, f=4 * (qc - 2) + i:
                                oproj_unit(bb, f) for i in range(4)]
                attn_chunk(b, qc)
                pump(len(fillers))   # flush leftovers
            alltoall(b)
        for fc in range(KC):
            oproj_unit(3, fc)

    nc.compile()
    return nc


def _make_in_maps(x, Wq, bq, Wk, bk, Wv, bv, Wo, bo):
    bf = ml_dtypes.bfloat16
    xT = np.ascontiguousarray(
        x.transpose(2, 0, 1).reshape(C, B * T)).astype(bf)
    WqT8 = (Wq.T / 8.0).astype(bf)
    WkT = Wk.T.astype(bf)
    WvT = Wv.T.astype(bf)
    WoT = np.ascontiguousarray(Wo.T.astype(bf))
    bq8 = (bq / 8.0).astype(np.float32)
    bo_f = (bo + Wo @ bv).astype(np.float32)
    bo8 = np.ascontiguousarray(bo_f.reshape(KC, P).T)
    kv = np.arange(P)[:, None]
    q = np.arange(CH)[None, :]
    msk = np.stack([(P * r + kv <= q) for r in range(4)], axis=1).astype(bf)
    msk = np.ascontiguousarray(msk)
    in_maps = []
    for c in range(8):
        sl = slice(P * c, P * (c + 1))
        in_maps.append({
            "xT": xT,
            "wq": np.ascontiguousarray(WqT8[:, sl]),
            "wk": np.ascontiguousarray(WkT[:, sl]),
            "wv": np.ascontiguousarray(WvT[:, sl]),
            "wo": WoT,
            "bq": np.ascontiguousarray(bq8[sl, None]),
            "bk": np.ascontiguousarray(bk[sl, None].astype(np.float32)),
            "bo": bo8,
            "mask": msk,
        })
    return in_maps


def kernel(x, Wq, bq, Wk, bk, Wv, bv, Wo, bo):
    from concourse.bass_utils import run_bass_kernel_spmd

    x = np.asarray(x, np.float32)
    Wq = np.asarray(Wq, np.float32); bq = np.asarray(bq, np.float32)
    Wk = np.asarray(Wk, np.float32); bk = np.asarray(bk, np.float32)
    Wv = np.asarray(Wv, np.float32); bv = np.asarray(bv, np.float32)
    Wo = np.asarray(Wo, np.float32); bo = np.asarray(bo, np.float32)

    if "nc" not in _CACHE:
        _CACHE["nc"] = _build()
    nc = _CACHE["nc"]

    in_maps = _make_in_maps(x, Wq, bq, Wk, bk, Wv, bv, Wo, bo)
    res = run_bass_kernel_spmd(nc, in_maps, core_ids=list(range(8)))
    outf = np.empty((B, T, C), np.float32)
    for c in range(8):
        o = res.results[c]["out"]            # (C, B*TS) transposed
        for b in range(B):
            outf[b, TS * c : TS * (c + 1), :] = o[:, TS * b : TS * (b + 1)].T
    return outf


# revision 12
# speedup vs baseline: 1.2578x; 1.1381x over previous
"""Causal multi-head attention (B=4, T=2048, C=1024, 16 heads) on 8 TRN2 cores.

Tensor-parallel over heads: core c owns heads 2c, 2c+1 (128 features) and
computes Q/K/V projections + attention for those heads over ALL 4 batches.
Causal work uses exact extents (q-chunk 512, kv extent 512*(qc+1)); for the
four diagonal kv blocks only the columns right of the diagonal are computed
(W = 512-128r), so just one 128x128 tril strip needs masking. Every core runs
the same instruction stream (SPMD) - only weight slices differ.

After attention, a per-batch 512KB AllToAll redistributes context from
feature-sharded to token-sharded; each core then runs the output projection
for its 256-token slice of each batch (contract over the full 1024 features).
Output projections are deferred (batch 0's into batch 3's attention, batches
1-2's to the tail) so the final collective is hidden behind PE work.

On-device layout is transposed ([feature, token]) except V (token-major, as
the ctx matmul's stationary operand). Softmax denominators come from a ones
column appended to V (l at PSUM partition 64); the two l rows are DMA-hopped
to partitions 0/32 of one tile, reciprocal'd in a single wide DVE op, and
broadcast across partitions with contract-1 PE matmuls. bk is dropped
entirely (softmax is invariant to a per-query logit offset); bv is folded
into the output bias (bo + Wo@bv); the 1/8 score scale into Wq/bq.

bf16 data path with fp32 PSUM accumulation. The ctx matmul for kv block n is
emitted after the scores for block n+1 so the PE never waits on the exp; Q/K/V
projections for batch b+1 are interleaved into batch b's attention as filler.
"""

import numpy as np
import ml_dtypes

B, T, C, NH, D = 4, 2048, 1024, 16, 64
P = 128
KC = C // P          # 8 contraction chunks
CH = 512             # q-chunk / projection token-chunk size
NCHUNK = T // CH     # 4 chunks per batch
TS = 256             # per-core token slice of each batch (T/8)

_CACHE = {}


def _build():
    import concourse.bacc as bacc
    import concourse.tile as tile
    import concourse.mybir as mybir
    from concourse.bass import ts, ds

    f32 = mybir.dt.float32
    bf16 = mybir.dt.bfloat16
    EXP = mybir.ActivationFunctionType.Exp
    MUL = mybir.AluOpType.mult
    ADD = mybir.AluOpType.add

    nc = bacc.Bacc("TRN2", target_bir_lowering=False, debug=False, num_devices=8)

    def din(name, shape, dt=bf16):
        return nc.dram_tensor(name, list(shape), dt, kind="ExternalInput").ap()

    xT = din("xT", (C, B * T))          # x^T, all batches (token = b*T + t)
    wq = din("wq", (C, P))              # (Wq/8)^T columns for this core's heads
    wk = din("wk", (C, P))
    wv = din("wv", (C, P))
    wo = din("wo", (C, C))              # full Wo^T
    bq = din("bq", (P, 1), f32)         # bq/8 slice
    bo = din("bo", (P, KC), f32)        # (bo + Wo@bv) chunked [128, 8]
    mask = din("mask", (P, P))          # 128x128 tril strip {0,1}
    out = nc.dram_tensor("out", [C, B * TS], f32, kind="ExternalOutput").ap()

    xT_v = xT.rearrange("(k p) t -> p k t", p=P)
    wq_v = wq.rearrange("(k p) m -> p k m", p=P)
    wk_v = wk.rearrange("(k p) m -> p k m", p=P)
    wv_v = wv.rearrange("(k p) m -> p k m", p=P)
    wo_v = wo.rearrange("(k p) m -> p k m", p=P)
    out_v = out.rearrange("(k p) t -> p k t", p=P)

    NBLK = T // P      # 16 kv blocks of 128 per batch

    from contextlib import ExitStack
    with ExitStack() as ctx:
        tc = ctx.enter_context(tile.TileContext(nc))

        consts = ctx.enter_context(tc.tile_pool(name="consts", bufs=1))
        wpool = ctx.enter_context(tc.tile_pool(name="w", bufs=1))
        xpool = ctx.enter_context(tc.tile_pool(name="x", bufs=3))
        qkpool = ctx.enter_context(tc.tile_pool(name="qk", bufs=2))
        vpool = ctx.enter_context(tc.tile_pool(name="v", bufs=2))
        ptpool = ctx.enter_context(tc.tile_pool(name="pt", bufs=4))
        lpool = ctx.enter_context(tc.tile_pool(name="l", bufs=2))
        cspool = ctx.enter_context(tc.tile_pool(name="cs", bufs=4))
        gpool = ctx.enter_context(tc.tile_pool(name="g", bufs=4))
        opool = ctx.enter_context(tc.tile_pool(name="o", bufs=2))
        psum = ctx.enter_context(tc.tile_pool(name="psum", bufs=2, space="PSUM"))
        dram = ctx.enter_context(tc.tile_pool(name="dram", bufs=2, space="DRAM"))

        # ---- constants ----
        bq_sb = consts.tile([P, 1], f32)
        bo_sb = consts.tile([P, KC], f32)
        msk_sb = consts.tile([P, P], bf16)
        nc.sync.dma_start(bq_sb[:], bq)
        nc.sync.dma_start(bo_sb[:], bo)
        nc.sync.dma_start(msk_sb[:], mask)
        # broadcast stationaries [1,...,1,0] living at partitions 0 and 32
        # (pairing with the hopped 1/l rows)
        pvec = consts.tile([33, D + 1], bf16)
        nc.vector.memset(pvec[0:1, 0:D], 1.0)
        nc.vector.memset(pvec[0:1, D : D + 1], 0.0)
        nc.vector.memset(pvec[32:33, 0:D], 1.0)
        nc.vector.memset(pvec[32:33, D : D + 1], 0.0)

        wq_sb = wpool.tile([P, KC, P], bf16, tag="wq", name="wq_sb")
        wk_sb = wpool.tile([P, KC, P], bf16, tag="wk", name="wk_sb")
        wv_sb = wpool.tile([P, KC, P], bf16, tag="wv", name="wv_sb")
        wo_sb = wpool.tile([P, KC, C], bf16, tag="wo", name="wo_sb")
        nc.sync.dma_start(wq_sb[:], wq_v)
        nc.sync.dma_start(wk_sb[:], wk_v)
        nc.sync.dma_start(wv_sb[:], wv_v)

        # ---- per-batch state ----
        QT = {}   # b -> [128, T] bf16 (2 heads x 64d on partitions)
        KT = {}
        VS = {}   # b -> [128, NBLK, 2, 65] token-major V (+ones col at 64)
        GC = {}   # b -> [128, KC, TS] gathered full-feature ctx
        BIN = {}  # b -> DRAM alltoall input bounce
        BOUT = {}
        XTT = {}

        def proj_chunk_q(b, t):
            """Tokens [CH*t, CH*t+CH) of batch b -> QT chunk (+x tile DMA)."""
            if b not in QT:
                QT[b] = qkpool.tile([P, T], bf16, tag="qt", name=f"qt{b}")
                KT[b] = qkpool.tile([P, T], bf16, tag="kt", name=f"kt{b}")
                VS[b] = vpool.tile([P, NBLK, 2, D + 1], bf16, tag="v",
                                   name=f"v{b}")
                nc.vector.memset(VS[b][:, :, :, D : D + 1], 1.0)
            xt = xpool.tile([P, KC, CH], bf16, tag="x", name=f"x{b}{t}")
            nc.sync.dma_start(xt[:], xT_v[:, :, ds(T * b + CH * t, CH)])
            XTT[(b, t)] = xt
            ps = psum.tile([P, CH], f32, tag="mix", name=f"pq{b}{t}")
            for k in range(KC):
                nc.tensor.matmul(ps[:], wq_sb[:, k, :], xt[:, k, :],
                                 start=(k == 0), stop=(k == KC - 1))
            nc.vector.tensor_tensor(QT[b][:, ds(CH * t, CH)], ps[:],
                                    bq_sb.to_broadcast((P, CH)), ADD)

        def proj_chunk_k(b, t):
            xt = XTT[(b, t)]
            ps = psum.tile([P, CH], f32, tag="mix", name=f"pk{b}{t}")
            for k in range(KC):
                nc.tensor.matmul(ps[:], wk_sb[:, k, :], xt[:, k, :],
                                 start=(k == 0), stop=(k == KC - 1))
            nc.vector.tensor_copy(KT[b][:, ds(CH * t, CH)], ps[:])

        def proj_chunk_v(b, t):
            xt = XTT.pop((b, t))
            for i in range(4):
                blk = 4 * t + i
                ps = psum.tile([P, P], f32, tag="mix", name=f"pv{b}{blk}")
                for k in range(KC):
                    nc.tensor.matmul(ps[:], xt[:, k, ts(i, P)], wv_sb[:, k, :],
                                     start=(k == 0), stop=(k == KC - 1))
                nc.vector.tensor_copy(
                    VS[b][:, blk, :, 0:D],
                    ps.rearrange("t (h d) -> t h d", d=D))

        def oproj_unit(b, fc):
            """Output features [128*fc, 128*fc+128) for batch b's token slice."""
            g = GC[b]
            ps = psum.tile([P, TS], f32, tag="mix", name=f"po{b}{fc}")
            for k in range(KC):
                nc.tensor.matmul(ps[:], wo_sb[:, k, ts(fc, P)], g[:, k, :],
                                 start=(k == 0), stop=(k == KC - 1))
            st = opool.tile([P, TS], f32, tag="o", name=f"o{b}{fc}")
            nc.vector.tensor_tensor(st[:], ps[:],
                                    bo_sb[:, fc : fc + 1].to_broadcast((P, TS)),
                                    ADD)
            nc.sync.dma_start(out_v[:, fc, ds(TS * b, TS)], st[:])

        # ---- filler pump ----
        fillers = []
        fi = [0]

        def pump(n):
            k = 0
            while k < n and fi[0] < len(fillers):
                fillers[fi[0]]()
                fi[0] += 1
                k += 1

        def attn_chunk(b, qc):
            """Attention for q rows [CH*qc, CH*qc+CH), kv [0, CH*(qc+1)).

            Diagonal kv blocks (r = blk-4qc >= 0) compute only the W=512-128r
            rightmost columns; the 128-wide strip at the left of that window
            gets the tril mask. ctx matmuls lag scores by one block.
            """
            nkv = 4 * (qc + 1)
            ctx_ps = [psum.tile([D + 1, CH], f32, tag="ctx",
                                name=f"cx{b}{qc}{h}") for h in range(2)]
            prev = None   # (blk, pt, off, W)

            def emit_ctx(blk, pt, off, W):
                for hh in range(2):
                    nc.tensor.matmul(
                        ctx_ps[hh][:, ds(off, W)],
                        VS[b][:, blk, hh, :],
                        pt[:, hh, 0:W],
                        start=(blk == 0), stop=(blk == nkv - 1))

            for blk in range(nkv):
                r = blk - 4 * qc
                off = max(r, 0) * P
                W = CH - off
                st = psum.tile([P, 2, CH], f32, tag="st", name=f"st{b}{qc}{blk}")
                for hh in range(2):
                    nc.tensor.matmul(
                        st[:, hh, 0:W],
                        KT[b][ds(D * hh, D), ds(P * blk, P)],
                        QT[b][ds(D * hh, D), ds(CH * qc + off, W)],
                        start=True, stop=True)
                if prev is not None:
                    emit_ctx(*prev)
                    pump(1)
                pt = ptpool.tile([P, 2, CH], bf16, tag="pt",
                                 name=f"pt{b}{qc}{blk}")
                nc.scalar.activation(pt[:, :, 0:W], st[:, :, 0:W], EXP)
                if r >= 0:   # mask the 128-wide strip at the diagonal
                    nc.vector.tensor_tensor(
                        pt[:, :, 0:P], pt[:, :, 0:P],
                        msk_sb.rearrange("p (u q) -> p u q", u=1)
                              .to_broadcast((P, 2, P)), MUL)
                prev = (blk, pt, off, W)
            emit_ctx(*prev)
            # epilogue: ctx rows 0..63, l at row 64. Hop both l rows into one
            # tile (partitions 0 / 32), one wide reciprocal, then contract-1
            # broadcast matmuls.
            lrows = lpool.tile([33, CH], f32, tag="lr", name=f"lr{b}{qc}")
            nc.vector.tensor_copy(lrows[0:1, :], ctx_ps[0][D : D + 1, :])
            nc.vector.tensor_copy(lrows[32:33, :], ctx_ps[1][D : D + 1, :])
            linv = lpool.tile([33, CH], bf16, tag="li", name=f"li{b}{qc}")
            with nc.allow_low_precision(reason="1/l in bf16; ~0.2% rel err"):
                nc.vector.reciprocal(linv[:], lrows[:])
            pump(1)
            bc = [psum.tile([D + 1, CH], f32, tag="mix", name=f"bc{b}{qc}{h}")
                  for h in range(2)]
            for hh in range(2):
                nc.tensor.matmul(bc[hh][:], pvec[ds(32 * hh, 1), :],
                                 linv[ds(32 * hh, 1), :],
                                 start=True, stop=True)
            pump(1)
            for hh in range(2):
                bcs = cspool.tile([D, CH], bf16, tag="bcs",
                                  name=f"bcs{b}{qc}{hh}")
                nc.vector.tensor_copy(bcs[:], bc[hh][0:D, :])
                cs = cspool.tile([D, CH], bf16, tag="cs",
                                 name=f"cs{b}{qc}{hh}")
                nc.vector.tensor_tensor(cs[:], ctx_ps[hh][0:D, :],
                                        bcs[:], MUL)
                # ship straight into the alltoall input bounce
                nc.sync.dma_start(
                    BIN[b].rearrange("j p t -> p j t")
                         [ds(D * hh, D), ds(2 * qc, 2), :],
                    cs.rearrange("p (j t) -> p j t", t=TS))

        def alltoall(b):
            BOUT[b] = dram.tile([8, P, TS], bf16, tag="cout", name=f"co{b}")
            nc.gpsimd.collective_compute(
                "AllToAll", mybir.AluOpType.bypass,
                replica_groups=[list(range(8))],
                ins=[BIN[b].opt()], outs=[BOUT[b].opt()])
            GC[b] = gpool.tile([P, KC, TS], bf16, tag="g", name=f"g{b}")
            nc.sync.dma_start(GC[b][:], BOUT[b].rearrange("s p t -> p s t"))

        # ---- emission schedule ----
        proj_chunk_q(0, 0); proj_chunk_k(0, 0); proj_chunk_v(0, 0)
        nc.sync.dma_start(wo_sb[:], wo_v)
        for b in range(B):
            BIN[b] = dram.tile([8, P, TS], bf16, tag="cin", name=f"ci{b}")
            for qc in range(NCHUNK):
                fillers.clear(); fi[0] = 0
                if b == 0 and qc < 3:
                    fillers += [lambda t=qc + 1: proj_chunk_q(0, t),
                                lambda t=qc + 1: proj_chunk_k(0, t),
                                lambda t=qc + 1: proj_chunk_v(0, t)]
                if b < 3:
                    fillers += [lambda t=qc, bb=b + 1: proj_chunk_q(bb, t),
                                lambda t=qc, bb=b + 1: proj_chunk_k(bb, t),
                                lambda t=qc, bb=b + 1: proj_chunk_v(bb, t)]
                else:
                    # batch 0's output projection fills batch 3's attention
                    fillers += [lambda f=2 * qc + i: oproj_unit(0, f)
                                for i in range(2)]
                attn_chunk(b, qc)
                pump(len(fillers))   # flush leftovers
            alltoall(b)
        # tail: batches 1-2 output projections hide the last collective
        for b in (1, 2):
            for fc in range(KC):
                oproj_unit(b, fc)
        for fc in range(KC):
            oproj_unit(3, fc)

    nc.compile()
    return nc


def _make_in_maps(x, Wq, bq, Wk, bk, Wv, bv, Wo, bo):
    bf = ml_dtypes.bfloat16
    xT = np.ascontiguousarray(
        x.transpose(2, 0, 1).reshape(C, B * T)).astype(bf)
    WqT8 = (Wq.T / 8.0).astype(bf)
    WkT = Wk.T.astype(bf)
    WvT = Wv.T.astype(bf)
    WoT = np.ascontiguousarray(Wo.T.astype(bf))
    bq8 = (bq / 8.0).astype(np.float32)
    bo_f = (bo + Wo @ bv).astype(np.float32)
    bo8 = np.ascontiguousarray(bo_f.reshape(KC, P).T)
    kv = np.arange(P)[:, None]
    q = np.arange(P)[None, :]
    msk = np.ascontiguousarray((kv <= q).astype(bf))
    in_maps = []
    for c in range(8):
        sl = slice(P * c, P * (c + 1))
        in_maps.append({
            "xT": xT,
            "wq": np.ascontiguousarray(WqT8[:, sl]),
            "wk": np.ascontiguousarray(WkT[:, sl]),
            "wv": np.ascontiguousarray(WvT[:, sl]),
            "wo": WoT,
            "bq": np.ascontiguousarray(bq8[sl, None]),
            "bo": bo8,
            "mask": msk,
        })
    return in_maps


def kernel(x, Wq, bq, Wk, bk, Wv, bv, Wo, bo):
    from concourse.bass_utils import run_bass_kernel_spmd

    x = np.asarray(x, np.float32)
    Wq = np.asarray(Wq, np.float32); bq = np.asarray(bq, np.float32)
    Wk = np.asarray(Wk, np.float32); bk = np.asarray(bk, np.float32)
    Wv = np.asarray(Wv, np.float32); bv = np.asarray(bv, np.float32)
    Wo = np.asarray(Wo, np.float32); bo = np.asarray(bo, np.float32)

    if "nc" not in _CACHE:
        _CACHE["nc"] = _build()
    nc = _CACHE["nc"]

    in_maps = _make_in_maps(x, Wq, bq, Wk, bk, Wv, bv, Wo, bo)
    res = run_bass_kernel_spmd(nc, in_maps, core_ids=list(range(8)))
    outf = np.empty((B, T, C), np.float32)
    for c in range(8):
        o = res.results[c]["out"]            # (C, B*TS) transposed
        for b in range(B):
            outf[b, TS * c : TS * (c + 1), :] = o[:, TS * b : TS * (b + 1)].T
    return outf
